# revision 10
# baseline (speedup 1.0000x reference)
"""Trainium2 Bass kernel for a dense transformer encoder layer.

Problem: B=2, S=2048, D=1024, H=16 heads (dk=64), FFN d_ff=4096, fp32,
padding mask zeroes whole query rows of the attention probabilities.

Sharding: sequence-parallel over the 4096 (batch*seq) query rows — each of
the 8 cores owns 512 query rows (4 cores per batch element) and recomputes
K/V for its full batch from X^T (no cross-core communication).  All
matmuls run as float32r (1 cycle/row on the PE at moving-dim >= 256,
~1e-4 relative error vs fp32).

Layout strategy: scores are computed transposed (k on partitions, q on
free dim), softmax needs no max-subtraction (scores ~ N(0,1) after the
1/sqrt(dk) scale, exp cannot overflow), and the softmax denominator falls
out of the P@V matmul for free via a ones-column appended to V.  The
normalization (1/denom, broadcast along partitions) is done with a tiny
K=2 matmul per head pair.  The padding mask is applied to attn_out rows
(query rows live on partitions there, so it's a natural [P,1] broadcast).

Attention runs in two k-halves (k in [0,1024), [1024,2048)) so K^T and
the augmented V for only half the sequence are SBUF-resident at a time;
the context accumulates across halves in SBUF (pre-normalization, which
is exact because the no-max softmax is a plain sum).
"""

import numpy as np

import concourse.bass as bass
import concourse.mybir as mybir
from concourse import bacc
from concourse import bass_utils
from concourse.masks import make_identity
from concourse.tile import TileContext

F32 = mybir.dt.float32
F32R = mybir.dt.float32r
AF = mybir.ActivationFunctionType
OP = mybir.AluOpType

B, S, D, H, DKH, DFF = 2, 2048, 1024, 16, 64, 4096
EPS = 1e-6
NCORES = 8
Q = (B * S) // NCORES   # 512 query rows per core
NQT = Q // 128          # 4 query tiles
NDC = D // 128          # 8 contraction chunks of 128
HALF = S // 2           # 1024 keys per attention phase
NKT = HALF // 128       # 8 k-tiles per half
NFT = DFF // 128        # 32 FFN tiles
VP = DKH + 1            # 65: V columns per head incl. ones column


def _build():
    nc = bacc.Bacc(None)

    # Per-core tensors (contents differ per core, same shapes).
    xt = nc.declare_dram_parameter("xt", [D, S], F32R, isOutput=False)
    xtq = nc.declare_dram_parameter("xtq", [D, Q], F32R, isOutput=False)
    xq = nc.declare_dram_parameter("xq", [Q, D], F32, isOutput=False)
    keep = nc.declare_dram_parameter("keep", [128, NQT], F32, isOutput=False)
    # Shared weights. *b variants are host-preblocked into [tile, D, 128]
    # column blocks so each block DMAs contiguously.
    wqb = nc.declare_dram_parameter("wqb", [NDC, D, 128], F32R, isOutput=False)
    wkb = nc.declare_dram_parameter("wkb", [NDC, D, 128], F32R, isOutput=False)
    wv = nc.declare_dram_parameter("wv", [D, D], F32R, isOutput=False)
    wo = nc.declare_dram_parameter("wo", [D, D], F32R, isOutput=False)
    w1b = nc.declare_dram_parameter("w1b", [NFT, D, 128], F32R, isOutput=False)
    w2 = nc.declare_dram_parameter("w2", [DFF, D], F32R, isOutput=False)
    b1m = nc.declare_dram_parameter("b1m", [128, NFT], F32, isOutput=False)
    gb1 = nc.declare_dram_parameter("gb1", [128, D], F32, isOutput=False)
    bb1 = nc.declare_dram_parameter("bb1", [128, D], F32, isOutput=False)
    gb2 = nc.declare_dram_parameter("gb2", [128, D], F32, isOutput=False)
    bb2 = nc.declare_dram_parameter("bb2", [128, D], F32, isOutput=False)
    b2b = nc.declare_dram_parameter("b2b", [128, D], F32, isOutput=False)
    out = nc.declare_dram_parameter("out", [Q, D], F32, isOutput=True)

    with TileContext(nc) as tc:
        with tc.tile_pool(name="constp", bufs=1) as constp:
            # ones64: K=1 matmuls broadcast each head's 1/denom over its 64
            # partitions; sliced at the denominator's base partition so lhsT
            # and rhs base partitions match.
            ones_f = constp.tile([128, 128], F32, name="ones_f")
            nc.vector.memset(ones_f[:], 1.0)
            # memset cannot legally produce f32r; a DVE copy rounds
            ones64 = constp.tile([128, 128], F32R, name="ones64")
            nc.vector.tensor_copy(ones64[:], ones_f[:])
            epsb = constp.tile([128, 1], F32, name="epsb")
            nc.vector.memset(epsb[:], EPS)

            ctxn = [constp.tile([128, Q], F32R, name=f"ctxn{t}", tag=f"ctxn{t}")
                    for t in range(NDC)]

            if True:
                with tc.tile_pool(name="qtp", bufs=1) as qtp:
                    qt_sb = [qtp.tile([128, Q], F32R, name=f"qts{t}", tag=f"qts{t}")
                             for t in range(NDC)]
                    # ---- Q^T = (Wq col-block)^T @ X_q^T, scaled later in exp ----
                    with tc.tile_pool(name="xtqp", bufs=1) as xtqp, \
                         tc.tile_pool(name="wstr", bufs=2) as wstr, \
                         tc.tile_pool(name="psA", bufs=3, space="PSUM") as psA:
                        xtq_sb = [xtqp.tile([128, Q], F32R, name=f"xtqs{c}", tag=f"xtqs{c}")
                                  for c in range(NDC)]
                        for c in range(NDC):
                            nc.sync.dma_start(out=xtq_sb[c][:],
                                              in_=xtq[c * 128:(c + 1) * 128, :])
                        for t in range(NDC):
                            wcb = wstr.tile([128, NDC * 128], F32R, name="wcb", tag="wcb")
                            nc.sync.dma_start(
                                out=wcb[:].rearrange("p (c m) -> p c m", m=128),
                                in_=wqb[t].rearrange("(c p) m -> p c m", p=128))
                            ps = psA.tile([128, Q], F32, name="psq", tag="psq")
                            for c in range(NDC):
                                nc.tensor.matmul(
                                    ps[:], wcb[:, c * 128:(c + 1) * 128], xtq_sb[c][:],
                                    start=(c == 0), stop=(c == NDC - 1))
                            nc.vector.tensor_copy(qt_sb[t][:], ps[:])

                    # ---- attention over two k-halves ----
                    with tc.tile_pool(name="ctxap", bufs=1) as ctxap:
                        ctx_acc = [ctxap.tile([128, Q], F32, name=f"ctxa{t}", tag=f"ctxa{t}")
                                   for t in range(NDC)]
                        # head h -> partition 32*(h%4) (legal SBUF start
                        # partitions are 0/32/64/96), free offset (h//4)*Q
                        den_acc = ctxap.tile([128, 8 * Q], F32, name="den_acc")

                        def dsl(t_, h):
                            # partitions {0, 64} only: the 1/denom broadcast
                            # matmul then uses tile_position (0,0)/(64,0),
                            # configs walrus accepts
                            return t_[64 * (h % 2):64 * (h % 2) + 1,
                                      (h // 2) * Q:(h // 2 + 1) * Q]

                        for hf in range(2):
                            koff = hf * HALF
                            with tc.tile_pool(name="kvp", bufs=1) as kvp:
                                kt_sb = [kvp.tile([128, HALF], F32R, name=f"kts{t}", tag=f"kts{t}")
                                         for t in range(NDC)]
                                v_sb = [kvp.tile([128, H * VP], F32R, name=f"vs{t}", tag=f"vs{t}")
                                        for t in range(NKT)]
                                with tc.tile_pool(name="xthp", bufs=1) as xthp, \
                                     tc.tile_pool(name="psB", bufs=3, space="PSUM") as psB:
                                    # this half's X^T columns only (32KB/partition)
                                    xth = [xthp.tile([128, HALF], F32R,
                                                     name=f"xth{c}", tag=f"xth{c}")
                                           for c in range(NDC)]
                                    for c in range(NDC):
                                        nc.sync.dma_start(
                                            out=xth[c][:],
                                            in_=xt[c * 128:(c + 1) * 128,
                                                   koff:koff + HALF])
                                    # K^T tiles: [dk-pair, k] = Wk-block^T @ X^T
                                    with tc.tile_pool(name="wstrK", bufs=2) as wstrK:
                                        for t in range(NDC):
                                            wcb = wstrK.tile([128, NDC * 128], F32R,
                                                             name="wcbk", tag="wcbk")
                                            nc.sync.dma_start(
                                                out=wcb[:].rearrange("p (c m) -> p c m", m=128),
                                                in_=wkb[t].rearrange("(c p) m -> p c m", p=128))
                                            for n in range(HALF // 512):
                                                ps = psB.tile([128, 512], F32, name="psk", tag="psk")
                                                for c in range(NDC):
                                                    nc.tensor.matmul(
                                                        ps[:],
                                                        wcb[:, c * 128:(c + 1) * 128],
                                                        xth[c][:, n * 512:(n + 1) * 512],
                                                        start=(c == 0), stop=(c == NDC - 1))
                                                nc.vector.tensor_copy(
                                                    kt_sb[t][:, n * 512:(n + 1) * 512], ps[:])
                                    # V tiles: [k, dv] = X-block^T @ Wv, written into
                                    # the 65-strided augmented layout; ones columns set
                                    # once per tile.
                                    for t in range(NKT):
                                        vr = v_sb[t][:].rearrange("p (h c) -> p h c", c=VP)
                                        nc.vector.tensor_copy(vr[:, :, DKH],
                                                              ones_f[:, 0:H])
                                    with tc.tile_pool(name="wstrV", bufs=1) as wstrV:
                                        for n in range(2):
                                            wvc = [wstrV.tile([128, 512], F32R,
                                                              name=f"wvc{c}", tag=f"wvc{c}")
                                                   for c in range(NDC)]
                                            for c in range(NDC):
                                                nc.sync.dma_start(
                                                    out=wvc[c][:],
                                                    in_=wv[c * 128:(c + 1) * 128,
                                                           n * 512:(n + 1) * 512])
                                            for t in range(NKT):
                                                ps = psB.tile([128, 512], F32, name="psv", tag="psv")
                                                for c in range(NDC):
                                                    nc.tensor.matmul(
                                                        ps[:], xth[c][:, t * 128:(t + 1) * 128],
                                                        wvc[c][:],
                                                        start=(c == 0), stop=(c == NDC - 1))
                                                vr = v_sb[t][:].rearrange("p (h c) -> p h c", c=VP)
                                                nc.vector.tensor_copy(
                                                    vr[:, n * 8:(n + 1) * 8, 0:DKH],
                                                    ps[:].rearrange("p (h c) -> p h c", c=DKH))

                                # per-head streaming attention for this half
                                with tc.tile_pool(name="expp", bufs=4) as expp, \
                                     tc.tile_pool(name="psS", bufs=4, space="PSUM") as psS, \
                                     tc.tile_pool(name="psC", bufs=2, space="PSUM") as psC:
                                    for h in range(H):
                                        t, sub = h // 2, h % 2
                                        cps = psC.tile([VP, Q], F32, name="cps", tag="cps")
                                        for kt in range(NKT):
                                            sps = psS.tile([128, Q], F32, name="sps", tag="sps")
                                            nc.tensor.matmul(
                                                sps[:],
                                                kt_sb[t][sub * 64:(sub + 1) * 64,
                                                         kt * 128:(kt + 1) * 128],
                                                qt_sb[t][sub * 64:(sub + 1) * 64, :],
                                                start=True, stop=True)
                                            ex = expp.tile([128, Q], F32R, name="ex", tag="ex")
                                            nc.scalar.activation(ex[:], sps[:], AF.Exp,
                                                                 scale=0.125)
                                            nc.tensor.matmul(
                                                cps[:],
                                                v_sb[kt][:, h * VP:(h + 1) * VP],
                                                ex[:],
                                                start=(kt == 0), stop=(kt == NKT - 1))
                                        dst = ctx_acc[t][sub * 64:(sub + 1) * 64, :]
                                        if hf == 0:
                                            nc.vector.tensor_copy(dst, cps[0:DKH, :])
                                            nc.vector.tensor_copy(dsl(den_acc, h),
                                                                  cps[DKH:VP, :])
                                        else:
                                            nc.vector.tensor_add(dst, dst, cps[0:DKH, :])
                                            nc.vector.tensor_add(dsl(den_acc, h),
                                                                 dsl(den_acc, h),
                                                                 cps[DKH:VP, :])

                        # normalize: ctxn = ctx_acc * (1/denom) per head
                        with tc.tile_pool(name="rcpp", bufs=1) as rcpp, \
                             tc.tile_pool(name="psR", bufs=2, space="PSUM") as psR:
                            rcp = rcpp.tile([128, 8 * Q], F32R, name="rcp")
                            with nc.allow_low_precision(
                                    reason="softmax 1/denom feeds an f32r matmul"):
                                for h in range(H):
                                    nc.vector.reciprocal(dsl(rcp, h),
                                                         dsl(den_acc, h))
                            for t in range(NDC):
                                for sub in range(2):
                                    h = 2 * t + sub
                                    base = 64 * (h % 2)
                                    rb = psR.tile([128, Q], F32, name="rb", tag="rb")
                                    nc.tensor.matmul(
                                        rb[:], ones64[base:base + 1, :],
                                        dsl(rcp, h), start=True, stop=True)
                                    sl = slice(sub * 64, (sub + 1) * 64)
                                    nc.vector.tensor_mul(ctxn[t][sl, :],
                                                         ctx_acc[t][sl, :],
                                                         rb[sl, :])

            # ---- W_O projection + mask + LN1 + residual -> x1; transpose x1 ----
            with tc.tile_pool(name="postp", bufs=1) as postp:
                x1 = [postp.tile([128, D], F32, name=f"x1_{i}", tag=f"x1_{i}")
                      for i in range(NQT)]
                x1t = [postp.tile([128, Q], F32R, name=f"x1t{c}", tag=f"x1t{c}")
                       for c in range(NDC)]
                with tc.tile_pool(name="lnp", bufs=2) as lnp, \
                     tc.tile_pool(name="lncp", bufs=1) as lncp, \
                     tc.tile_pool(name="wstr3", bufs=3) as wstr3, \
                     tc.tile_pool(name="psW", bufs=4, space="PSUM") as psW:
                    keep_sb = lncp.tile([128, NQT], F32, name="keep_sb")
                    nc.sync.dma_start(out=keep_sb[:], in_=keep[:, :])
                    gb1_sb = lncp.tile([128, D], F32, name="gb1_sb")
                    nc.sync.dma_start(out=gb1_sb[:], in_=gb1[:, :])
                    bb1_sb = lncp.tile([128, D], F32, name="bb1_sb")
                    nc.sync.dma_start(out=bb1_sb[:], in_=bb1[:, :])
                    xq_sb = [lncp.tile([128, D], F32, name=f"xqs{i}", tag=f"xqs{i}")
                             for i in range(NQT)]
                    for i in range(NQT):
                        nc.sync.dma_start(out=xq_sb[i][:],
                                          in_=xq[i * 128:(i + 1) * 128, :])
                    woc = {}
                    for n in range(2):
                        for c in range(NDC):
                            woc[(n, c)] = wstr3.tile([128, 512], F32R, name="woc",
                                                     tag=f"woc{n}_{c}", bufs=1)
                            nc.sync.dma_start(
                                out=woc[(n, c)][:],
                                in_=wo[c * 128:(c + 1) * 128, n * 512:(n + 1) * 512])
                    for i in range(NQT):
                        ao = lnp.tile([128, D], F32, name="ao", tag="ao")
                        for n in range(2):
                            ps = psW.tile([128, 512], F32, name="psw", tag="psw")
                            for c in range(NDC):
                                nc.tensor.matmul(
                                    ps[:], ctxn[c][:, i * 128:(i + 1) * 128],
                                    woc[(n, c)][:],
                                    start=(c == 0), stop=(c == NDC - 1))
                            # psum -> sbuf with padding mask fused in
                            nc.vector.tensor_scalar_mul(
                                ao[:, n * 512:(n + 1) * 512], ps[:],
                                keep_sb[:, i:i + 1])
                        # LayerNorm(ao) * gamma1 + beta1 + xq -> x1
                        stat = lnp.tile([128, 4], F32, name="stat", tag="stat")
                        cent = lnp.tile([128, D], F32, name="cent", tag="cent")
                        sq = lnp.tile([128, D], F32, name="sq", tag="sq")
                        nc.vector.tensor_reduce(stat[:, 0:1], ao[:], mybir.AxisListType.X, OP.add)
                        nc.vector.tensor_scalar_mul(stat[:, 1:2], stat[:, 0:1], 1.0 / D)
                        nc.vector.tensor_scalar_sub(cent[:], ao[:], stat[:, 1:2])
                        nc.vector.scalar_tensor_tensor(
                            sq[:], ao[:], stat[:, 1:2], cent[:],
                            op0=OP.subtract, op1=OP.mult, accum_out=stat[:, 2:3])
                        nc.scalar.activation(stat[:, 3:4], stat[:, 2:3], AF.Sqrt,
                                             bias=epsb[:, 0:1], scale=1.0 / D)
                        nc.vector.reciprocal(stat[:, 0:1], stat[:, 3:4])
                        t1 = lnp.tile([128, D], F32, name="t1", tag="t1")
                        nc.vector.scalar_tensor_tensor(
                            t1[:], cent[:], stat[:, 0:1], gb1_sb[:],
                            op0=OP.mult, op1=OP.mult)
                        t2 = lnp.tile([128, D], F32, name="t2", tag="t2")
                        nc.vector.tensor_add(t2[:], xq_sb[i][:], bb1_sb[:])
                        nc.vector.tensor_add(x1[i][:], t1[:], t2[:])
                    # transpose x1 -> x1t (f32r) for the FFN contraction
                    identity = lncp.tile([128, 128], F32, name="identity")
                    make_identity(nc, identity[:])
                    for i in range(NQT):
                        for c in range(NDC):
                            ps = psW.tile([128, 128], F32, name="pst", tag="pst")
                            nc.tensor.transpose(ps[:], x1[i][:, c * 128:(c + 1) * 128],
                                                identity[:])
                            nc.vector.tensor_copy(x1t[c][:, i * 128:(i + 1) * 128], ps[:])

                # ---- FFN: hT = relu(W1-block^T @ x1T + b1); out accumulates hT^T @ W2 ----
                with tc.tile_pool(name="hp", bufs=1) as hp:
                    ht = [hp.tile([128, Q], F32R, name=f"ht{t}", tag=f"ht{t}")
                          for t in range(NFT)]
                    with tc.tile_pool(name="wstr4", bufs=2) as wstr4, \
                         tc.tile_pool(name="bp", bufs=1) as bp, \
                         tc.tile_pool(name="psF", bufs=3, space="PSUM") as psF:
                        b1_sb = bp.tile([128, NFT], F32, name="b1_sb")
                        nc.sync.dma_start(out=b1_sb[:], in_=b1m[:, :])
                        for t in range(NFT):
                            wcb = wstr4.tile([128, NDC * 128], F32R, name="wcb1", tag="wcb1")
                            nc.sync.dma_start(
                                out=wcb[:].rearrange("p (c m) -> p c m", m=128),
                                in_=w1b[t].rearrange("(c p) m -> p c m", p=128))
                            ps = psF.tile([128, Q], F32, name="psh", tag="psh")
                            for c in range(NDC):
                                nc.tensor.matmul(
                                    ps[:], wcb[:, c * 128:(c + 1) * 128], x1t[c][:],
                                    start=(c == 0), stop=(c == NDC - 1))
                            nc.scalar.activation(ht[t][:], ps[:], AF.Relu,
                                                 bias=b1_sb[:, t:t + 1])

                    with tc.tile_pool(name="wstr5", bufs=4) as wstr5, \
                         tc.tile_pool(name="ln2p", bufs=2) as ln2p, \
                         tc.tile_pool(name="ln2c", bufs=1) as ln2c, \
                         tc.tile_pool(name="psO", bufs=1, space="PSUM") as psO:
                        gb2_sb = ln2c.tile([128, D], F32, name="gb2_sb")
                        nc.sync.dma_start(out=gb2_sb[:], in_=gb2[:, :])
                        bb2_sb = ln2c.tile([128, D], F32, name="bb2_sb")
                        nc.sync.dma_start(out=bb2_sb[:], in_=bb2[:, :])
                        b2b_sb = ln2c.tile([128, D], F32, name="b2b_sb")
                        nc.sync.dma_start(out=b2b_sb[:], in_=b2b[:, :])
                        fo = [ln2p.tile([128, D], F32, name=f"fo{i}", tag=f"fo{i}", bufs=1)
                              for i in range(NQT)]
                        for n in range(2):
                            pss = [psO.tile([128, 512], F32, name=f"pso{i}", tag=f"pso{i}{n}")
                                   for i in range(NQT)]
                            for t in range(NFT):
                                w2c = wstr5.tile([128, 512], F32R, name="w2c", tag="w2c")
                                nc.sync.dma_start(
                                    out=w2c[:],
                                    in_=w2[t * 128:(t + 1) * 128, n * 512:(n + 1) * 512])
                                for i in range(NQT):
                                    nc.tensor.matmul(
                                        pss[i][:], ht[t][:, i * 128:(i + 1) * 128],
                                        w2c[:],
                                        start=(t == 0), stop=(t == NFT - 1))
                            for i in range(NQT):
                                nc.vector.tensor_add(fo[i][:, n * 512:(n + 1) * 512],
                                                     pss[i][:],
                                                     b2b_sb[:, n * 512:(n + 1) * 512])
                        for i in range(NQT):
                            stat = ln2p.tile([128, 4], F32, name="stat2", tag="stat2")
                            cent = ln2p.tile([128, D], F32, name="cent2", tag="cent2")
                            sq = ln2p.tile([128, D], F32, name="sq2", tag="sq2")
                            nc.vector.tensor_reduce(stat[:, 0:1], fo[i][:],
                                                    mybir.AxisListType.X, OP.add)
                            nc.vector.tensor_scalar_mul(stat[:, 1:2], stat[:, 0:1], 1.0 / D)
                            nc.vector.tensor_scalar_sub(cent[:], fo[i][:], stat[:, 1:2])
                            nc.vector.scalar_tensor_tensor(
                                sq[:], fo[i][:], stat[:, 1:2], cent[:],
                                op0=OP.subtract, op1=OP.mult, accum_out=stat[:, 2:3])
                            nc.scalar.activation(stat[:, 3:4], stat[:, 2:3], AF.Sqrt,
                                                 bias=epsb[:, 0:1], scale=1.0 / D)
                            nc.vector.reciprocal(stat[:, 0:1], stat[:, 3:4])
                            t1 = ln2p.tile([128, D], F32, name="t1b", tag="t1b")
                            nc.vector.scalar_tensor_tensor(
                                t1[:], cent[:], stat[:, 0:1], gb2_sb[:],
                                op0=OP.mult, op1=OP.mult)
                            t2 = ln2p.tile([128, D], F32, name="t2b", tag="t2b")
                            nc.vector.tensor_add(t2[:], x1[i][:], bb2_sb[:])
                            xo = ln2p.tile([128, D], F32, name="xo", tag="xo")
                            nc.vector.tensor_add(xo[:], t1[:], t2[:])
                            nc.sync.dma_start(out=out[i * 128:(i + 1) * 128, :], in_=xo[:])

    nc.finalize()
    return nc


_NC = None


def _get_nc():
    global _NC
    if _NC is None:
        _NC = _build()
    return _NC


def _host_prep(batch_X, padding_mask, W_Q, W_K, W_V, W_O, W1, b1, W2, b2,
               gamma1, beta1, gamma2, beta2):
    """Build the 8 per-core input maps."""
    f = np.float32
    X = np.asarray(batch_X, f)
    pm = np.asarray(padding_mask)

    def colblocks(w, nt):
        # [D, nt*128] -> [nt, D, 128] contiguous column blocks
        return np.ascontiguousarray(
            np.asarray(w, f).reshape(w.shape[0], nt, 128).transpose(1, 0, 2))

    shared = {
        "wqb": colblocks(W_Q, NDC),
        "wkb": colblocks(W_K, NDC),
        "wv": np.ascontiguousarray(np.asarray(W_V, f)),
        "wo": np.ascontiguousarray(np.asarray(W_O, f)),
        "w1b": colblocks(W1, NFT),
        "w2": np.ascontiguousarray(np.asarray(W2, f)),
        "b1m": np.ascontiguousarray(np.asarray(b1, f).reshape(NFT, 128).T),
        "gb1": np.ascontiguousarray(
            np.broadcast_to(np.asarray(gamma1, f), (128, D))),
        "bb1": np.ascontiguousarray(
            np.broadcast_to(np.asarray(beta1, f), (128, D))),
        "gb2": np.ascontiguousarray(
            np.broadcast_to(np.asarray(gamma2, f), (128, D))),
        "bb2": np.ascontiguousarray(
            np.broadcast_to(np.asarray(beta2, f), (128, D))),
        "b2b": np.ascontiguousarray(
            np.broadcast_to(np.asarray(b2, f), (128, D))),
    }
    in_maps = []
    for core in range(NCORES):
        b = core // (NCORES // B)
        q0 = (core % (NCORES // B)) * Q
        xb = X[b]
        keep_f = (pm[b, q0:q0 + Q] != 0).astype(f)
        m = dict(shared)
        m["xt"] = np.ascontiguousarray(xb.T)
        m["xtq"] = np.ascontiguousarray(xb[q0:q0 + Q].T)
        m["xq"] = np.ascontiguousarray(xb[q0:q0 + Q])
        m["keep"] = np.ascontiguousarray(keep_f.reshape(NQT, 128).T)
        in_maps.append(m)
    return in_maps


def kernel(**inputs):
    nc = _get_nc()
    in_maps = _host_prep(**inputs)
    res = bass_utils.run_bass_kernel_spmd(nc, in_maps, list(range(NCORES)))
    out = np.empty((B, S, D), np.float32)
    for core in range(NCORES):
        b = core // (NCORES // B)
        q0 = (core % (NCORES // B)) * Q
        out[b, q0:q0 + Q] = res.results[core]["out"]
    return out


# revision 12
# speedup vs baseline: 1.0178x; 1.0178x over previous
"""Trainium2 Bass kernel for a dense transformer encoder layer.

Problem: B=2, S=2048, D=1024, H=16 heads (dk=64), FFN d_ff=4096, fp32,
padding mask zeroes whole query rows of the attention probabilities.

Sharding: sequence-parallel over the 4096 (batch*seq) query rows — each of
the 8 cores owns 512 query rows (4 cores per batch element) and recomputes
K/V for its full batch from X^T (no cross-core communication).  All
matmuls run as float32r (1 cycle/row on the PE at moving-dim >= 256,
~1e-4 relative error vs fp32).

Layout strategy: scores are computed transposed (k on partitions, q on
free dim), softmax needs no max-subtraction (scores ~ N(0,1) after the
1/sqrt(dk) scale, exp cannot overflow), and the softmax denominator falls
out of the P@V matmul for free via a ones-column appended to V.  The
normalization (1/denom, broadcast along partitions) is done with a tiny
K=2 matmul per head pair.  The padding mask is applied to attn_out rows
(query rows live on partitions there, so it's a natural [P,1] broadcast).

Attention runs in two k-halves (k in [0,1024), [1024,2048)) so K^T and
the augmented V for only half the sequence are SBUF-resident at a time;
the context accumulates across halves in SBUF (pre-normalization, which
is exact because the no-max softmax is a plain sum).
"""

import numpy as np

import concourse.bass as bass
import concourse.mybir as mybir
from concourse import bacc
from concourse import bass_utils
from concourse.masks import make_identity
from concourse.tile import TileContext

F32 = mybir.dt.float32
F32R = mybir.dt.float32r
AF = mybir.ActivationFunctionType
OP = mybir.AluOpType

B, S, D, H, DKH, DFF = 2, 2048, 1024, 16, 64, 4096
EPS = 1e-6
NCORES = 8
Q = (B * S) // NCORES   # 512 query rows per core
NQT = Q // 128          # 4 query tiles
NDC = D // 128          # 8 contraction chunks of 128
HALF = S // 2           # 1024 keys per attention phase
NKT = HALF // 128       # 8 k-tiles per half
NFT = DFF // 128        # 32 FFN tiles
VP = DKH + 1            # 65: V columns per head incl. ones column


def _build():
    nc = bacc.Bacc(None)

    # Per-core tensors (contents differ per core, same shapes).
    xt = nc.declare_dram_parameter("xt", [D, S], F32R, isOutput=False)
    xtq = nc.declare_dram_parameter("xtq", [D, Q], F32R, isOutput=False)
    xq = nc.declare_dram_parameter("xq", [Q, D], F32, isOutput=False)
    keep = nc.declare_dram_parameter("keep", [128, NQT], F32, isOutput=False)
    # Shared weights. *b variants are host-preblocked into [tile, D, 128]
    # column blocks so each block DMAs contiguously.
    wqb = nc.declare_dram_parameter("wqb", [NDC, 128, D], F32R, isOutput=False)
    wkb = nc.declare_dram_parameter("wkb", [NDC, 128, D], F32R, isOutput=False)
    wv = nc.declare_dram_parameter("wv", [D, D], F32R, isOutput=False)
    wo = nc.declare_dram_parameter("wo", [D, D], F32R, isOutput=False)
    w1b = nc.declare_dram_parameter("w1b", [NFT, 128, D], F32R, isOutput=False)
    w2 = nc.declare_dram_parameter("w2", [DFF, D], F32R, isOutput=False)
    b1m = nc.declare_dram_parameter("b1m", [128, NFT], F32, isOutput=False)
    gb1 = nc.declare_dram_parameter("gb1", [128, D], F32, isOutput=False)
    bb1 = nc.declare_dram_parameter("bb1", [128, D], F32, isOutput=False)
    gb2 = nc.declare_dram_parameter("gb2", [128, D], F32, isOutput=False)
    bb2 = nc.declare_dram_parameter("bb2", [128, D], F32, isOutput=False)
    b2b = nc.declare_dram_parameter("b2b", [128, D], F32, isOutput=False)
    out = nc.declare_dram_parameter("out", [Q, D], F32, isOutput=True)

    with TileContext(nc) as tc:
        with tc.tile_pool(name="constp", bufs=1) as constp:
            # ones64: K=1 matmuls broadcast each head's 1/denom over its 64
            # partitions; sliced at the denominator's base partition so lhsT
            # and rhs base partitions match.
            ones_f = constp.tile([128, 128], F32, name="ones_f")
            nc.vector.memset(ones_f[:], 1.0)
            # memset cannot legally produce f32r; a DVE copy rounds
            ones64 = constp.tile([128, 128], F32R, name="ones64")
            nc.vector.tensor_copy(ones64[:], ones_f[:])
            epsb = constp.tile([128, 1], F32, name="epsb")
            nc.vector.memset(epsb[:], EPS)

            ctxn = [constp.tile([128, Q], F32R, name=f"ctxn{t}", tag=f"ctxn{t}")
                    for t in range(NDC)]

            if True:
                with tc.tile_pool(name="qtp", bufs=1) as qtp:
                    qt_sb = [qtp.tile([128, Q], F32R, name=f"qts{t}", tag=f"qts{t}")
                             for t in range(NDC)]
                    # ---- Q^T = (Wq col-block)^T @ X_q^T, scaled later in exp ----
                    with tc.tile_pool(name="xtqp", bufs=1) as xtqp, \
                         tc.tile_pool(name="wstr", bufs=2) as wstr, \
                         tc.tile_pool(name="psA", bufs=3, space="PSUM") as psA:
                        xtq_sb = [xtqp.tile([128, Q], F32R, name=f"xtqs{c}", tag=f"xtqs{c}")
                                  for c in range(NDC)]
                        for c in range(NDC):
                            nc.sync.dma_start(out=xtq_sb[c][:],
                                              in_=xtq[c * 128:(c + 1) * 128, :])
                        for t in range(NDC):
                            wcb = wstr.tile([128, NDC * 128], F32R, name="wcb", tag="wcb")
                            nc.sync.dma_start(
                                out=wcb[:], in_=wqb[t])
                            ps = psA.tile([128, Q], F32, name="psq", tag="psq")
                            for c in range(NDC):
                                nc.tensor.matmul(
                                    ps[:], wcb[:, c * 128:(c + 1) * 128], xtq_sb[c][:],
                                    start=(c == 0), stop=(c == NDC - 1))
                            nc.vector.tensor_copy(qt_sb[t][:], ps[:])

                    # ---- attention over two k-halves ----
                    with tc.tile_pool(name="ctxap", bufs=1) as ctxap:
                        ctx_acc = [ctxap.tile([128, Q], F32, name=f"ctxa{t}", tag=f"ctxa{t}")
                                   for t in range(NDC)]
                        # head h -> partition 32*(h%4) (legal SBUF start
                        # partitions are 0/32/64/96), free offset (h//4)*Q
                        den_acc = ctxap.tile([128, 8 * Q], F32, name="den_acc")

                        def dsl(t_, h):
                            # partitions {0, 64} only: the 1/denom broadcast
                            # matmul then uses tile_position (0,0)/(64,0),
                            # configs walrus accepts
                            return t_[64 * (h % 2):64 * (h % 2) + 1,
                                      (h // 2) * Q:(h // 2 + 1) * Q]

                        for hf in range(2):
                            koff = hf * HALF
                            with tc.tile_pool(name="kvp", bufs=1) as kvp:
                                kt_sb = [kvp.tile([128, HALF], F32R, name=f"kts{t}", tag=f"kts{t}")
                                         for t in range(NDC)]
                                v_sb = [kvp.tile([128, H * VP], F32R, name=f"vs{t}", tag=f"vs{t}")
                                        for t in range(NKT)]
                                with tc.tile_pool(name="xthp", bufs=1) as xthp, \
                                     tc.tile_pool(name="psB", bufs=3, space="PSUM") as psB:
                                    # this half's X^T columns only (32KB/partition)
                                    xth = [xthp.tile([128, HALF], F32R,
                                                     name=f"xth{c}", tag=f"xth{c}")
                                           for c in range(NDC)]
                                    for c in range(NDC):
                                        nc.sync.dma_start(
                                            out=xth[c][:],
                                            in_=xt[c * 128:(c + 1) * 128,
                                                   koff:koff + HALF])
                                    # K^T tiles: [dk-pair, k] = Wk-block^T @ X^T
                                    with tc.tile_pool(name="wstrK", bufs=2) as wstrK:
                                        for t in range(NDC):
                                            wcb = wstrK.tile([128, NDC * 128], F32R,
                                                             name="wcbk", tag="wcbk")
                                            nc.sync.dma_start(
                                                out=wcb[:], in_=wkb[t])
                                            for n in range(HALF // 512):
                                                ps = psB.tile([128, 512], F32, name="psk", tag="psk")
                                                for c in range(NDC):
                                                    nc.tensor.matmul(
                                                        ps[:],
                                                        wcb[:, c * 128:(c + 1) * 128],
                                                        xth[c][:, n * 512:(n + 1) * 512],
                                                        start=(c == 0), stop=(c == NDC - 1))
                                                nc.vector.tensor_copy(
                                                    kt_sb[t][:, n * 512:(n + 1) * 512], ps[:])
                                    # V tiles: [k, dv] = X-block^T @ Wv, written into
                                    # the 65-strided augmented layout; ones columns set
                                    # once per tile.
                                    for t in range(NKT):
                                        vr = v_sb[t][:].rearrange("p (h c) -> p h c", c=VP)
                                        nc.vector.tensor_copy(vr[:, :, DKH],
                                                              ones_f[:, 0:H])
                                    with tc.tile_pool(name="wstrV", bufs=1) as wstrV:
                                        for n in range(2):
                                            wvc = [wstrV.tile([128, 512], F32R,
                                                              name=f"wvc{c}", tag=f"wvc{c}")
                                                   for c in range(NDC)]
                                            for c in range(NDC):
                                                nc.sync.dma_start(
                                                    out=wvc[c][:],
                                                    in_=wv[c * 128:(c + 1) * 128,
                                                           n * 512:(n + 1) * 512])
                                            for t in range(NKT):
                                                ps = psB.tile([128, 512], F32, name="psv", tag="psv")
                                                for c in range(NDC):
                                                    nc.tensor.matmul(
                                                        ps[:], xth[c][:, t * 128:(t + 1) * 128],
                                                        wvc[c][:],
                                                        start=(c == 0), stop=(c == NDC - 1))
                                                vr = v_sb[t][:].rearrange("p (h c) -> p h c", c=VP)
                                                nc.vector.tensor_copy(
                                                    vr[:, n * 8:(n + 1) * 8, 0:DKH],
                                                    ps[:].rearrange("p (h c) -> p h c", c=DKH))

                                # per-head streaming attention for this half
                                with tc.tile_pool(name="expp", bufs=4) as expp, \
                                     tc.tile_pool(name="psS", bufs=4, space="PSUM") as psS, \
                                     tc.tile_pool(name="psC", bufs=2, space="PSUM") as psC:
                                    for h in range(H):
                                        t, sub = h // 2, h % 2
                                        cps = psC.tile([VP, Q], F32, name="cps", tag="cps")
                                        for kt in range(NKT):
                                            sps = psS.tile([128, Q], F32, name="sps", tag="sps")
                                            nc.tensor.matmul(
                                                sps[:],
                                                kt_sb[t][sub * 64:(sub + 1) * 64,
                                                         kt * 128:(kt + 1) * 128],
                                                qt_sb[t][sub * 64:(sub + 1) * 64, :],
                                                start=True, stop=True)
                                            ex = expp.tile([128, Q], F32R, name="ex", tag="ex")
                                            nc.scalar.activation(ex[:], sps[:], AF.Exp,
                                                                 scale=0.125)
                                            nc.tensor.matmul(
                                                cps[:],
                                                v_sb[kt][:, h * VP:(h + 1) * VP],
                                                ex[:],
                                                start=(kt == 0), stop=(kt == NKT - 1))
                                        dst = ctx_acc[t][sub * 64:(sub + 1) * 64, :]
                                        if hf == 0:
                                            nc.vector.tensor_copy(dst, cps[0:DKH, :])
                                            nc.vector.tensor_copy(dsl(den_acc, h),
                                                                  cps[DKH:VP, :])
                                        else:
                                            # fold this half's contribution and
                                            # normalize immediately — the slow
                                            # reciprocal hides under PE work of
                                            # subsequent heads
                                            nc.vector.tensor_add(dst, dst, cps[0:DKH, :])
                                            nc.vector.tensor_add(dsl(den_acc, h),
                                                                 dsl(den_acc, h),
                                                                 cps[DKH:VP, :])
                                            rcp = expp.tile([128, Q], F32R,
                                                            name="rcph", tag="rcph",
                                                            bufs=2)
                                            with nc.allow_low_precision(
                                                    reason="softmax 1/denom feeds f32r matmul"):
                                                nc.vector.reciprocal(
                                                    rcp[64 * (h % 2):64 * (h % 2) + 1, :],
                                                    dsl(den_acc, h))
                                            rb = psS.tile([128, Q], F32,
                                                          name="rbn", tag="rbn", bufs=2)
                                            nc.tensor.matmul(
                                                rb[:],
                                                ones64[64 * (h % 2):64 * (h % 2) + 1, :],
                                                rcp[64 * (h % 2):64 * (h % 2) + 1, :],
                                                start=True, stop=True)
                                            nc.vector.tensor_mul(
                                                ctxn[t][sub * 64:(sub + 1) * 64, :],
                                                dst, rb[sub * 64:(sub + 1) * 64, :])


            # ---- W_O projection + mask + LN1 + residual -> x1; transpose x1 ----
            with tc.tile_pool(name="postp", bufs=1) as postp:
                x1 = [postp.tile([128, D], F32, name=f"x1_{i}", tag=f"x1_{i}")
                      for i in range(NQT)]
                x1t = [postp.tile([128, Q], F32R, name=f"x1t{c}", tag=f"x1t{c}")
                       for c in range(NDC)]
                with tc.tile_pool(name="lnp", bufs=2) as lnp, \
                     tc.tile_pool(name="lncp", bufs=1) as lncp, \
                     tc.tile_pool(name="wstr3", bufs=3) as wstr3, \
                     tc.tile_pool(name="psW", bufs=4, space="PSUM") as psW:
                    keep_sb = lncp.tile([128, NQT], F32, name="keep_sb")
                    nc.sync.dma_start(out=keep_sb[:], in_=keep[:, :])
                    gb1_sb = lncp.tile([128, D], F32, name="gb1_sb")
                    nc.sync.dma_start(out=gb1_sb[:], in_=gb1[:, :])
                    bb1_sb = lncp.tile([128, D], F32, name="bb1_sb")
                    nc.sync.dma_start(out=bb1_sb[:], in_=bb1[:, :])
                    xq_sb = [lncp.tile([128, D], F32, name=f"xqs{i}", tag=f"xqs{i}")
                             for i in range(NQT)]
                    for i in range(NQT):
                        nc.sync.dma_start(out=xq_sb[i][:],
                                          in_=xq[i * 128:(i + 1) * 128, :])
                    woc = {}
                    for n in range(2):
                        for c in range(NDC):
                            woc[(n, c)] = wstr3.tile([128, 512], F32R, name="woc",
                                                     tag=f"woc{n}_{c}", bufs=1)
                            nc.sync.dma_start(
                                out=woc[(n, c)][:],
                                in_=wo[c * 128:(c + 1) * 128, n * 512:(n + 1) * 512])
                    for i in range(NQT):
                        ao = lnp.tile([128, D], F32, name="ao", tag="ao")
                        for n in range(2):
                            ps = psW.tile([128, 512], F32, name="psw", tag="psw")
                            for c in range(NDC):
                                nc.tensor.matmul(
                                    ps[:], ctxn[c][:, i * 128:(i + 1) * 128],
                                    woc[(n, c)][:],
                                    start=(c == 0), stop=(c == NDC - 1))
                            # psum -> sbuf with padding mask fused in
                            nc.vector.tensor_scalar_mul(
                                ao[:, n * 512:(n + 1) * 512], ps[:],
                                keep_sb[:, i:i + 1])
                        # LayerNorm(ao) * gamma1 + beta1 + xq -> x1
                        stat = lnp.tile([128, 4], F32, name="stat", tag="stat")
                        cent = lnp.tile([128, D], F32, name="cent", tag="cent")
                        sq = lnp.tile([128, D], F32, name="sq", tag="sq")
                        nc.vector.tensor_reduce(stat[:, 0:1], ao[:], mybir.AxisListType.X, OP.add)
                        nc.vector.tensor_scalar_mul(stat[:, 1:2], stat[:, 0:1], 1.0 / D)
                        nc.vector.tensor_scalar_sub(cent[:], ao[:], stat[:, 1:2])
                        nc.vector.scalar_tensor_tensor(
                            sq[:], ao[:], stat[:, 1:2], cent[:],
                            op0=OP.subtract, op1=OP.mult, accum_out=stat[:, 2:3])
                        nc.scalar.activation(stat[:, 3:4], stat[:, 2:3], AF.Sqrt,
                                             bias=epsb[:, 0:1], scale=1.0 / D)
                        nc.vector.reciprocal(stat[:, 0:1], stat[:, 3:4])
                        t1 = lnp.tile([128, D], F32, name="t1", tag="t1")
                        nc.vector.scalar_tensor_tensor(
                            t1[:], cent[:], stat[:, 0:1], gb1_sb[:],
                            op0=OP.mult, op1=OP.mult)
                        t2 = lnp.tile([128, D], F32, name="t2", tag="t2")
                        nc.vector.tensor_add(t2[:], xq_sb[i][:], bb1_sb[:])
                        nc.vector.tensor_add(x1[i][:], t1[:], t2[:])
                    # transpose x1 -> x1t (f32r) for the FFN contraction
                    identity = lncp.tile([128, 128], F32, name="identity")
                    make_identity(nc, identity[:])
                    for i in range(NQT):
                        for c in range(NDC):
                            ps = psW.tile([128, 128], F32, name="pst", tag="pst")
                            nc.tensor.transpose(ps[:], x1[i][:, c * 128:(c + 1) * 128],
                                                identity[:])
                            nc.vector.tensor_copy(x1t[c][:, i * 128:(i + 1) * 128], ps[:])

                # ---- FFN: hT = relu(W1-block^T @ x1T + b1); out accumulates hT^T @ W2 ----
                with tc.tile_pool(name="hp", bufs=1) as hp:
                    ht = [hp.tile([128, Q], F32R, name=f"ht{t}", tag=f"ht{t}")
                          for t in range(NFT)]
                    with tc.tile_pool(name="wstr4", bufs=2) as wstr4, \
                         tc.tile_pool(name="bp", bufs=1) as bp, \
                         tc.tile_pool(name="psF", bufs=3, space="PSUM") as psF:
                        b1_sb = bp.tile([128, NFT], F32, name="b1_sb")
                        nc.sync.dma_start(out=b1_sb[:], in_=b1m[:, :])
                        for t in range(NFT):
                            wcb = wstr4.tile([128, NDC * 128], F32R, name="wcb1", tag="wcb1")
                            nc.sync.dma_start(
                                out=wcb[:], in_=w1b[t])
                            ps = psF.tile([128, Q], F32, name="psh", tag="psh")
                            for c in range(NDC):
                                nc.tensor.matmul(
                                    ps[:], wcb[:, c * 128:(c + 1) * 128], x1t[c][:],
                                    start=(c == 0), stop=(c == NDC - 1))
                            nc.scalar.activation(ht[t][:], ps[:], AF.Relu,
                                                 bias=b1_sb[:, t:t + 1])

                    with tc.tile_pool(name="wstr5", bufs=4) as wstr5, \
                         tc.tile_pool(name="ln2p", bufs=2) as ln2p, \
                         tc.tile_pool(name="ln2c", bufs=1) as ln2c, \
                         tc.tile_pool(name="psO", bufs=1, space="PSUM") as psO:
                        gb2_sb = ln2c.tile([128, D], F32, name="gb2_sb")
                        nc.sync.dma_start(out=gb2_sb[:], in_=gb2[:, :])
                        bb2_sb = ln2c.tile([128, D], F32, name="bb2_sb")
                        nc.sync.dma_start(out=bb2_sb[:], in_=bb2[:, :])
                        b2b_sb = ln2c.tile([128, D], F32, name="b2b_sb")
                        nc.sync.dma_start(out=b2b_sb[:], in_=b2b[:, :])
                        fo = [ln2p.tile([128, D], F32, name=f"fo{i}", tag=f"fo{i}", bufs=1)
                              for i in range(NQT)]
                        for n in range(2):
                            pss = [psO.tile([128, 512], F32, name=f"pso{i}", tag=f"pso{i}{n}")
                                   for i in range(NQT)]
                            for t in range(NFT):
                                w2c = wstr5.tile([128, 512], F32R, name="w2c", tag="w2c")
                                nc.sync.dma_start(
                                    out=w2c[:],
                                    in_=w2[t * 128:(t + 1) * 128, n * 512:(n + 1) * 512])
                                for i in range(NQT):
                                    nc.tensor.matmul(
                                        pss[i][:], ht[t][:, i * 128:(i + 1) * 128],
                                        w2c[:],
                                        start=(t == 0), stop=(t == NFT - 1))
                            for i in range(NQT):
                                nc.vector.tensor_add(fo[i][:, n * 512:(n + 1) * 512],
                                                     pss[i][:],
                                                     b2b_sb[:, n * 512:(n + 1) * 512])
                        for i in range(NQT):
                            stat = ln2p.tile([128, 4], F32, name="stat2", tag="stat2")
                            cent = ln2p.tile([128, D], F32, name="cent2", tag="cent2")
                            sq = ln2p.tile([128, D], F32, name="sq2", tag="sq2")
                            nc.vector.tensor_reduce(stat[:, 0:1], fo[i][:],
                                                    mybir.AxisListType.X, OP.add)
                            nc.vector.tensor_scalar_mul(stat[:, 1:2], stat[:, 0:1], 1.0 / D)
                            nc.vector.tensor_scalar_sub(cent[:], fo[i][:], stat[:, 1:2])
                            nc.vector.scalar_tensor_tensor(
                                sq[:], fo[i][:], stat[:, 1:2], cent[:],
                                op0=OP.subtract, op1=OP.mult, accum_out=stat[:, 2:3])
                            nc.scalar.activation(stat[:, 3:4], stat[:, 2:3], AF.Sqrt,
                                                 bias=epsb[:, 0:1], scale=1.0 / D)
                            nc.vector.reciprocal(stat[:, 0:1], stat[:, 3:4])
                            t1 = ln2p.tile([128, D], F32, name="t1b", tag="t1b")
                            nc.vector.scalar_tensor_tensor(
                                t1[:], cent[:], stat[:, 0:1], gb2_sb[:],
                                op0=OP.mult, op1=OP.mult)
                            t2 = ln2p.tile([128, D], F32, name="t2b", tag="t2b")
                            nc.vector.tensor_add(t2[:], x1[i][:], bb2_sb[:])
                            xo = ln2p.tile([128, D], F32, name="xo", tag="xo")
                            nc.vector.tensor_add(xo[:], t1[:], t2[:])
                            nc.sync.dma_start(out=out[i * 128:(i + 1) * 128, :], in_=xo[:])

    nc.finalize()
    return nc


_NC = None


def _get_nc():
    global _NC
    if _NC is None:
        _NC = _build()
    return _NC


def _host_prep(batch_X, padding_mask, W_Q, W_K, W_V, W_O, W1, b1, W2, b2,
               gamma1, beta1, gamma2, beta2):
    """Build the 8 per-core input maps."""
    f = np.float32
    X = np.asarray(batch_X, f)
    pm = np.asarray(padding_mask)

    def colblocks(w, nt):
        # [D, nt*128] -> [nt, 128, D]: block t, partition p holds the
        # contraction row for W[:, t*128+m] laid out [c, m] contiguously,
        # so each DMA partition line is one contiguous 4KB read
        nd = w.shape[0] // 128
        return np.ascontiguousarray(
            np.asarray(w, f).reshape(nd, 128, nt, 128).transpose(2, 1, 0, 3)
        ).reshape(nt, 128, w.shape[0])

    shared = {
        "wqb": colblocks(W_Q, NDC),
        "wkb": colblocks(W_K, NDC),
        "wv": np.ascontiguousarray(np.asarray(W_V, f)),
        "wo": np.ascontiguousarray(np.asarray(W_O, f)),
        "w1b": colblocks(W1, NFT),
        "w2": np.ascontiguousarray(np.asarray(W2, f)),
        "b1m": np.ascontiguousarray(np.asarray(b1, f).reshape(NFT, 128).T),
        "gb1": np.ascontiguousarray(
            np.broadcast_to(np.asarray(gamma1, f), (128, D))),
        "bb1": np.ascontiguousarray(
            np.broadcast_to(np.asarray(beta1, f), (128, D))),
        "gb2": np.ascontiguousarray(
            np.broadcast_to(np.asarray(gamma2, f), (128, D))),
        "bb2": np.ascontiguousarray(
            np.broadcast_to(np.asarray(beta2, f), (128, D))),
        "b2b": np.ascontiguousarray(
            np.broadcast_to(np.asarray(b2, f), (128, D))),
    }
    in_maps = []
    for core in range(NCORES):
        b = core // (NCORES // B)
        q0 = (core % (NCORES // B)) * Q
        xb = X[b]
        keep_f = (pm[b, q0:q0 + Q] != 0).astype(f)
        m = dict(shared)
        m["xt"] = np.ascontiguousarray(xb.T)
        m["xtq"] = np.ascontiguousarray(xb[q0:q0 + Q].T)
        m["xq"] = np.ascontiguousarray(xb[q0:q0 + Q])
        m["keep"] = np.ascontiguousarray(keep_f.reshape(NQT, 128).T)
        in_maps.append(m)
    return in_maps


def kernel(**inputs):
    nc = _get_nc()
    in_maps = _host_prep(**inputs)
    res = bass_utils.run_bass_kernel_spmd(nc, in_maps, list(range(NCORES)))
    out = np.empty((B, S, D), np.float32)
    for core in range(NCORES):
        b = core // (NCORES // B)
        q0 = (core % (NCORES // B)) * Q
        out[b, q0:q0 + Q] = res.results[core]["out"]
    return out


# revision 14
# speedup vs baseline: 1.3966x; 1.3721x over previous
"""Trainium2 Bass kernel for a dense transformer encoder layer.

Problem: B=2, S=2048, D=1024, H=16 heads (dk=64), FFN d_ff=4096, fp32,
padding mask zeroes whole query rows of the attention probabilities.

Sharding: sequence-parallel over the 4096 (batch*seq) query rows — each of
the 8 cores owns 512 query rows (4 cores per batch element) and recomputes
K/V for its full batch from X^T (no cross-core communication).  All
matmuls run as float32r (1 cycle/row on the PE at moving-dim >= 256,
~1e-4 relative error vs fp32).

Layout strategy: scores are computed transposed (k on partitions, q on
free dim), softmax needs no max-subtraction (scores ~ N(0,1) after the
1/sqrt(dk) scale, exp cannot overflow), and the softmax denominator falls
out of the P@V matmul for free via a ones-column appended to V.  The
normalization (1/denom, broadcast along partitions) is done with a tiny
K=2 matmul per head pair.  The padding mask is applied to attn_out rows
(query rows live on partitions there, so it's a natural [P,1] broadcast).

Attention runs in two k-halves (k in [0,1024), [1024,2048)) so K^T and
the augmented V for only half the sequence are SBUF-resident at a time;
the context accumulates across halves in SBUF (pre-normalization, which
is exact because the no-max softmax is a plain sum).
"""

import numpy as np

import concourse.bass as bass
import concourse.mybir as mybir
from concourse import bacc
from concourse import bass_utils
from concourse.masks import make_identity
from concourse.tile import TileContext

F32 = mybir.dt.float32
F32R = mybir.dt.float32r
AF = mybir.ActivationFunctionType
OP = mybir.AluOpType

B, S, D, H, DKH, DFF = 2, 2048, 1024, 16, 64, 4096
EPS = 1e-6
NCORES = 8
Q = (B * S) // NCORES   # 512 query rows per core
NQT = Q // 128          # 4 query tiles
NDC = D // 128          # 8 contraction chunks of 128
HALF = S // 2           # 1024 keys per attention phase
NKT = HALF // 128       # 8 k-tiles per half
NFT = DFF // 128        # 32 FFN tiles
VP = DKH + 1            # 65: V columns per head incl. ones column


def _build():
    nc = bacc.Bacc(None)

    # Per-core tensors (contents differ per core, same shapes).
    xt = nc.declare_dram_parameter("xt", [D, S], F32R, isOutput=False)
    xtq = nc.declare_dram_parameter("xtq", [D, Q], F32R, isOutput=False)
    xq = nc.declare_dram_parameter("xq", [Q, D], F32, isOutput=False)
    keep = nc.declare_dram_parameter("keep", [128, NQT], F32, isOutput=False)
    # Shared weights. *b variants are host-preblocked into [tile, D, 128]
    # column blocks so each block DMAs contiguously.
    wqb = nc.declare_dram_parameter("wqb", [NDC, 128, D], F32R, isOutput=False)
    wkb = nc.declare_dram_parameter("wkb", [NDC, 128, D], F32R, isOutput=False)
    wv = nc.declare_dram_parameter("wv", [D, D], F32R, isOutput=False)
    wo = nc.declare_dram_parameter("wo", [D, D], F32R, isOutput=False)
    w1b = nc.declare_dram_parameter("w1b", [NFT, 128, D], F32R, isOutput=False)
    w2 = nc.declare_dram_parameter("w2", [DFF, D], F32R, isOutput=False)
    b1m = nc.declare_dram_parameter("b1m", [128, NFT], F32, isOutput=False)
    gb1 = nc.declare_dram_parameter("gb1", [128, D], F32, isOutput=False)
    bb1 = nc.declare_dram_parameter("bb1", [128, D], F32, isOutput=False)
    gb2 = nc.declare_dram_parameter("gb2", [128, D], F32, isOutput=False)
    bb2 = nc.declare_dram_parameter("bb2", [128, D], F32, isOutput=False)
    b2b = nc.declare_dram_parameter("b2b", [128, D], F32, isOutput=False)
    out = nc.declare_dram_parameter("out", [Q, D], F32, isOutput=True)

    with TileContext(nc) as tc:
        with tc.tile_pool(name="constp", bufs=1) as constp:
            # ones64: K=1 matmuls broadcast each head's 1/denom over its 64
            # partitions; sliced at the denominator's base partition so lhsT
            # and rhs base partitions match.
            ones_f = constp.tile([128, 128], F32, name="ones_f")
            nc.vector.memset(ones_f[:], 1.0)
            # memset cannot legally produce f32r; a DVE copy rounds
            ones64 = constp.tile([128, 128], F32R, name="ones64")
            nc.vector.tensor_copy(ones64[:], ones_f[:])
            epsb = constp.tile([128, 1], F32, name="epsb")
            nc.vector.memset(epsb[:], EPS)

            ctxn = [constp.tile([128, Q], F32R, name=f"ctxn{t}", tag=f"ctxn{t}")
                    for t in range(NDC)]

            if True:
                with tc.tile_pool(name="qtp", bufs=1) as qtp:
                    qt_sb = [qtp.tile([128, Q], F32R, name=f"qts{t}", tag=f"qts{t}")
                             for t in range(NDC)]
                    # ---- Q^T = (Wq col-block)^T @ X_q^T, scaled later in exp ----
                    with tc.tile_pool(name="xtqp", bufs=1) as xtqp, \
                         tc.tile_pool(name="wstr", bufs=2) as wstr, \
                         tc.tile_pool(name="psA", bufs=3, space="PSUM") as psA:
                        xtq_sb = [xtqp.tile([128, Q], F32R, name=f"xtqs{c}", tag=f"xtqs{c}")
                                  for c in range(NDC)]
                        for c in range(NDC):
                            nc.sync.dma_start(out=xtq_sb[c][:],
                                              in_=xtq[c * 128:(c + 1) * 128, :])
                        for t in range(NDC):
                            wcb = wstr.tile([128, NDC * 128], F32R, name="wcb", tag="wcb")
                            nc.sync.dma_start(
                                out=wcb[:], in_=wqb[t])
                            ps = psA.tile([128, Q], F32, name="psq", tag="psq")
                            for c in range(NDC):
                                nc.tensor.matmul(
                                    ps[:], wcb[:, c * 128:(c + 1) * 128], xtq_sb[c][:],
                                    start=(c == 0), stop=(c == NDC - 1))
                            nc.vector.tensor_copy(qt_sb[t][:], ps[:])

                    # ---- attention over two k-halves ----
                    with tc.tile_pool(name="ctxap", bufs=1) as ctxap:
                        ctx_acc = [ctxap.tile([128, Q], F32, name=f"ctxa{t}", tag=f"ctxa{t}")
                                   for t in range(NDC)]
                        # head h -> partition 32*(h%4) (legal SBUF start
                        # partitions are 0/32/64/96), free offset (h//4)*Q
                        den_acc = ctxap.tile([128, 8 * Q], F32, name="den_acc")

                        def dsl(t_, h):
                            # partitions {0, 64} only: the 1/denom broadcast
                            # matmul then uses tile_position (0,0)/(64,0),
                            # configs walrus accepts
                            return t_[64 * (h % 2):64 * (h % 2) + 1,
                                      (h // 2) * Q:(h // 2 + 1) * Q]

                        for hf in range(2):
                            koff = hf * HALF
                            with tc.tile_pool(name="kvp", bufs=1) as kvp:
                                kt_sb = [kvp.tile([128, HALF], F32R, name=f"kts{t}", tag=f"kts{t}")
                                         for t in range(NDC)]
                                v_sb = [kvp.tile([128, H * VP], F32R, name=f"vs{t}", tag=f"vs{t}")
                                        for t in range(NKT)]
                                with tc.tile_pool(name="xthp", bufs=1) as xthp, \
                                     tc.tile_pool(name="psB", bufs=3, space="PSUM") as psB:
                                    # this half's X^T columns only (32KB/partition)
                                    xth = [xthp.tile([128, HALF], F32R,
                                                     name=f"xth{c}", tag=f"xth{c}")
                                           for c in range(NDC)]
                                    for c in range(NDC):
                                        nc.sync.dma_start(
                                            out=xth[c][:],
                                            in_=xt[c * 128:(c + 1) * 128,
                                                   koff:koff + HALF])
                                    # K^T tiles: [dk-pair, k] = Wk-block^T @ X^T
                                    with tc.tile_pool(name="wstrK", bufs=2) as wstrK:
                                        for t in range(NDC):
                                            wcb = wstrK.tile([128, NDC * 128], F32R,
                                                             name="wcbk", tag="wcbk")
                                            nc.sync.dma_start(
                                                out=wcb[:], in_=wkb[t])
                                            for n in range(HALF // 512):
                                                ps = psB.tile([128, 512], F32, name="psk", tag="psk")
                                                for c in range(NDC):
                                                    nc.tensor.matmul(
                                                        ps[:],
                                                        wcb[:, c * 128:(c + 1) * 128],
                                                        xth[c][:, n * 512:(n + 1) * 512],
                                                        start=(c == 0), stop=(c == NDC - 1))
                                                nc.vector.tensor_copy(
                                                    kt_sb[t][:, n * 512:(n + 1) * 512], ps[:])
                                    # V tiles: [k, dv] = X-block^T @ Wv, written into
                                    # the 65-strided augmented layout; ones columns set
                                    # once per tile.
                                    for t in range(NKT):
                                        vr = v_sb[t][:].rearrange("p (h c) -> p h c", c=VP)
                                        nc.vector.tensor_copy(vr[:, :, DKH],
                                                              ones_f[:, 0:H])
                                    with tc.tile_pool(name="wstrV", bufs=1) as wstrV:
                                        for n in range(2):
                                            wvc = [wstrV.tile([128, 512], F32R,
                                                              name=f"wvc{c}", tag=f"wvc{c}")
                                                   for c in range(NDC)]
                                            for c in range(NDC):
                                                nc.sync.dma_start(
                                                    out=wvc[c][:],
                                                    in_=wv[c * 128:(c + 1) * 128,
                                                           n * 512:(n + 1) * 512])
                                            for t in range(NKT):
                                                ps = psB.tile([128, 512], F32, name="psv", tag="psv")
                                                for c in range(NDC):
                                                    nc.tensor.matmul(
                                                        ps[:], xth[c][:, t * 128:(t + 1) * 128],
                                                        wvc[c][:],
                                                        start=(c == 0), stop=(c == NDC - 1))
                                                vr = v_sb[t][:].rearrange("p (h c) -> p h c", c=VP)
                                                nc.vector.tensor_copy(
                                                    vr[:, n * 8:(n + 1) * 8, 0:DKH],
                                                    ps[:].rearrange("p (h c) -> p h c", c=DKH))

                                # per-pair streaming attention for this half:
                                # one [128, 2Q] scores psum (2 banks) + a single
                                # exp per (pair, k-tile) halves ACT dispatches
                                with tc.tile_pool(name="expp", bufs=4) as expp, \
                                     tc.tile_pool(name="psS", bufs=2, space="PSUM") as psS, \
                                     tc.tile_pool(name="psC", bufs=1, space="PSUM") as psC:
                                    for t in range(NDC):
                                        cps = [psC.tile([VP, Q], F32, name=f"cps{sb_}",
                                                        tag=f"cps{sb_}")
                                               for sb_ in range(2)]
                                        for kt in range(NKT):
                                            sps = psS.tile([128, 2 * Q], F32,
                                                           name="sps", tag="sps")
                                            for sub in range(2):
                                                nc.tensor.matmul(
                                                    sps[:, sub * Q:(sub + 1) * Q],
                                                    kt_sb[t][sub * 64:(sub + 1) * 64,
                                                             kt * 128:(kt + 1) * 128],
                                                    qt_sb[t][sub * 64:(sub + 1) * 64, :],
                                                    start=True, stop=True)
                                            ex = expp.tile([128, 2 * Q], F32R,
                                                           name="ex", tag="ex")
                                            nc.scalar.activation(ex[:], sps[:], AF.Exp,
                                                                 scale=0.125)
                                            for sub in range(2):
                                                h = 2 * t + sub
                                                nc.tensor.matmul(
                                                    cps[sub][:],
                                                    v_sb[kt][:, h * VP:(h + 1) * VP],
                                                    ex[:, sub * Q:(sub + 1) * Q],
                                                    start=(kt == 0), stop=(kt == NKT - 1))
                                        for sub in range(2):
                                            h = 2 * t + sub
                                            dst = ctx_acc[t][sub * 64:(sub + 1) * 64, :]
                                            if hf == 0:
                                                nc.vector.tensor_copy(dst, cps[sub][0:DKH, :])
                                                nc.vector.tensor_copy(dsl(den_acc, h),
                                                                      cps[sub][DKH:VP, :])
                                            else:
                                                # fold this half's contribution and
                                                # normalize immediately — the slow
                                                # reciprocal hides under PE work of
                                                # subsequent pairs
                                                nc.vector.tensor_add(dst, dst,
                                                                     cps[sub][0:DKH, :])
                                                nc.vector.tensor_add(dsl(den_acc, h),
                                                                     dsl(den_acc, h),
                                                                     cps[sub][DKH:VP, :])
                                                rcp = expp.tile([128, Q], F32R,
                                                                name="rcph", tag="rcph",
                                                                bufs=2)
                                                with nc.allow_low_precision(
                                                        reason="softmax 1/denom feeds f32r matmul"):
                                                    nc.vector.reciprocal(
                                                        rcp[64 * (h % 2):64 * (h % 2) + 1, :],
                                                        dsl(den_acc, h))
                                                rb = psS.tile([128, Q], F32,
                                                              name="rbn", tag="rbn")
                                                nc.tensor.matmul(
                                                    rb[:],
                                                    ones64[64 * (h % 2):64 * (h % 2) + 1, :],
                                                    rcp[64 * (h % 2):64 * (h % 2) + 1, :],
                                                    start=True, stop=True)
                                                nc.vector.tensor_mul(
                                                    ctxn[t][sub * 64:(sub + 1) * 64, :],
                                                    dst, rb[sub * 64:(sub + 1) * 64, :])

            # ---- W_O projection + mask + LN1 + residual -> x1; transpose x1 ----
            with tc.tile_pool(name="postp", bufs=1) as postp:
                x1 = [postp.tile([128, D], F32, name=f"x1_{i}", tag=f"x1_{i}")
                      for i in range(NQT)]
                x1t = [postp.tile([128, Q], F32R, name=f"x1t{c}", tag=f"x1t{c}")
                       for c in range(NDC)]
                # FFN-phase streaming pools opened early: fresh addresses (no
                # WAR against the WO/LN pools), so their weight DMAs prefetch
                # while the PE is still on W_O / transposes
                wstr4 = tc.alloc_tile_pool(name="wstr4", bufs=3)
                wstr5 = tc.alloc_tile_pool(name="wstr5", bufs=6)
                bp = tc.alloc_tile_pool(name="bp", bufs=1)
                ln2c = tc.alloc_tile_pool(name="ln2c", bufs=1)
                b1_sb = bp.tile([128, NFT], F32, name="b1_sb")
                nc.sync.dma_start(out=b1_sb[:], in_=b1m[:, :])
                gb2_sb = ln2c.tile([128, D], F32, name="gb2_sb")
                nc.sync.dma_start(out=gb2_sb[:], in_=gb2[:, :])
                bb2_sb = ln2c.tile([128, D], F32, name="bb2_sb")
                nc.sync.dma_start(out=bb2_sb[:], in_=bb2[:, :])
                b2b_sb = ln2c.tile([128, D], F32, name="b2b_sb")
                nc.sync.dma_start(out=b2b_sb[:], in_=b2b[:, :])
                with tc.tile_pool(name="lnp", bufs=2) as lnp, \
                     tc.tile_pool(name="lncp", bufs=1) as lncp, \
                     tc.tile_pool(name="wstr3", bufs=3) as wstr3, \
                     tc.tile_pool(name="psW", bufs=4, space="PSUM") as psW:
                    keep_sb = lncp.tile([128, NQT], F32, name="keep_sb")
                    nc.sync.dma_start(out=keep_sb[:], in_=keep[:, :])
                    gb1_sb = lncp.tile([128, D], F32, name="gb1_sb")
                    nc.sync.dma_start(out=gb1_sb[:], in_=gb1[:, :])
                    bb1_sb = lncp.tile([128, D], F32, name="bb1_sb")
                    nc.sync.dma_start(out=bb1_sb[:], in_=bb1[:, :])
                    xq_sb = [lncp.tile([128, D], F32, name=f"xqs{i}", tag=f"xqs{i}")
                             for i in range(NQT)]
                    for i in range(NQT):
                        nc.sync.dma_start(out=xq_sb[i][:],
                                          in_=xq[i * 128:(i + 1) * 128, :])
                    woc = {}
                    for n in range(2):
                        for c in range(NDC):
                            woc[(n, c)] = wstr3.tile([128, 512], F32R, name="woc",
                                                     tag=f"woc{n}_{c}", bufs=1)
                            nc.sync.dma_start(
                                out=woc[(n, c)][:],
                                in_=wo[c * 128:(c + 1) * 128, n * 512:(n + 1) * 512])
                    for i in range(NQT):
                        ao = lnp.tile([128, D], F32, name="ao", tag="ao")
                        for n in range(2):
                            ps = psW.tile([128, 512], F32, name="psw", tag="psw")
                            for c in range(NDC):
                                nc.tensor.matmul(
                                    ps[:], ctxn[c][:, i * 128:(i + 1) * 128],
                                    woc[(n, c)][:],
                                    start=(c == 0), stop=(c == NDC - 1))
                            # psum -> sbuf with padding mask fused in
                            nc.vector.tensor_scalar_mul(
                                ao[:, n * 512:(n + 1) * 512], ps[:],
                                keep_sb[:, i:i + 1])
                        # LayerNorm(ao) * gamma1 + beta1 + xq -> x1
                        stat = lnp.tile([128, 4], F32, name="stat", tag="stat")
                        cent = lnp.tile([128, D], F32, name="cent", tag="cent")
                        sq = lnp.tile([128, D], F32, name="sq", tag="sq")
                        nc.vector.tensor_reduce(stat[:, 0:1], ao[:], mybir.AxisListType.X, OP.add)
                        nc.vector.tensor_scalar_mul(stat[:, 1:2], stat[:, 0:1], 1.0 / D)
                        nc.vector.tensor_scalar_sub(cent[:], ao[:], stat[:, 1:2])
                        nc.vector.scalar_tensor_tensor(
                            sq[:], ao[:], stat[:, 1:2], cent[:],
                            op0=OP.subtract, op1=OP.mult, accum_out=stat[:, 2:3])
                        nc.scalar.activation(stat[:, 3:4], stat[:, 2:3], AF.Sqrt,
                                             bias=epsb[:, 0:1], scale=1.0 / D)
                        nc.vector.reciprocal(stat[:, 0:1], stat[:, 3:4])
                        t1 = lnp.tile([128, D], F32, name="t1", tag="t1")
                        nc.vector.scalar_tensor_tensor(
                            t1[:], cent[:], stat[:, 0:1], gb1_sb[:],
                            op0=OP.mult, op1=OP.mult)
                        t2 = lnp.tile([128, D], F32, name="t2", tag="t2")
                        nc.vector.tensor_add(t2[:], xq_sb[i][:], bb1_sb[:])
                        nc.vector.tensor_add(x1[i][:], t1[:], t2[:])
                    # transpose x1 -> x1t (f32r) for the FFN contraction
                    identity = lncp.tile([128, 128], F32, name="identity")
                    make_identity(nc, identity[:])
                    for i in range(NQT):
                        for c in range(NDC):
                            ps = psW.tile([128, 128], F32, name="pst", tag="pst")
                            nc.tensor.transpose(ps[:], x1[i][:, c * 128:(c + 1) * 128],
                                                identity[:])
                            nc.vector.tensor_copy(x1t[c][:, i * 128:(i + 1) * 128], ps[:])

                # ---- FFN: hT = relu(W1-block^T @ x1T + b1); out accumulates hT^T @ W2 ----
                with tc.tile_pool(name="hp", bufs=1) as hp:
                    ht = [hp.tile([128, Q], F32R, name=f"ht{t}", tag=f"ht{t}")
                          for t in range(NFT)]
                    with tc.tile_pool(name="psF", bufs=3, space="PSUM") as psF:
                        for t in range(NFT):
                            wcb = wstr4.tile([128, NDC * 128], F32R, name="wcb1", tag="wcb1")
                            nc.sync.dma_start(
                                out=wcb[:], in_=w1b[t])
                            ps = psF.tile([128, Q], F32, name="psh", tag="psh")
                            for c in range(NDC):
                                nc.tensor.matmul(
                                    ps[:], wcb[:, c * 128:(c + 1) * 128], x1t[c][:],
                                    start=(c == 0), stop=(c == NDC - 1))
                            nc.scalar.activation(ht[t][:], ps[:], AF.Relu,
                                                 bias=b1_sb[:, t:t + 1])

                    with tc.tile_pool(name="ln2p", bufs=2) as ln2p, \
                         tc.tile_pool(name="psO", bufs=1, space="PSUM") as psO:
                        fo = [ln2p.tile([128, D], F32, name=f"fo{i}", tag=f"fo{i}", bufs=1)
                              for i in range(NQT)]
                        for n in range(2):
                            pss = [psO.tile([128, 512], F32, name=f"pso{i}", tag=f"pso{i}{n}")
                                   for i in range(NQT)]
                            for t in range(NFT):
                                w2c = wstr5.tile([128, 512], F32R, name="w2c", tag="w2c")
                                nc.sync.dma_start(
                                    out=w2c[:],
                                    in_=w2[t * 128:(t + 1) * 128, n * 512:(n + 1) * 512])
                                for i in range(NQT):
                                    nc.tensor.matmul(
                                        pss[i][:], ht[t][:, i * 128:(i + 1) * 128],
                                        w2c[:],
                                        start=(t == 0), stop=(t == NFT - 1))
                            for i in range(NQT):
                                nc.vector.tensor_add(fo[i][:, n * 512:(n + 1) * 512],
                                                     pss[i][:],
                                                     b2b_sb[:, n * 512:(n + 1) * 512])
                        for i in range(NQT):
                            stat = ln2p.tile([128, 4], F32, name="stat2", tag="stat2")
                            cent = ln2p.tile([128, D], F32, name="cent2", tag="cent2")
                            sq = ln2p.tile([128, D], F32, name="sq2", tag="sq2")
                            nc.vector.tensor_reduce(stat[:, 0:1], fo[i][:],
                                                    mybir.AxisListType.X, OP.add)
                            nc.vector.tensor_scalar_mul(stat[:, 1:2], stat[:, 0:1], 1.0 / D)
                            nc.vector.tensor_scalar_sub(cent[:], fo[i][:], stat[:, 1:2])
                            nc.vector.scalar_tensor_tensor(
                                sq[:], fo[i][:], stat[:, 1:2], cent[:],
                                op0=OP.subtract, op1=OP.mult, accum_out=stat[:, 2:3])
                            nc.scalar.activation(stat[:, 3:4], stat[:, 2:3], AF.Sqrt,
                                                 bias=epsb[:, 0:1], scale=1.0 / D)
                            nc.vector.reciprocal(stat[:, 0:1], stat[:, 3:4])
                            t1 = ln2p.tile([128, D], F32, name="t1b", tag="t1b")
                            nc.vector.scalar_tensor_tensor(
                                t1[:], cent[:], stat[:, 0:1], gb2_sb[:],
                                op0=OP.mult, op1=OP.mult)
                            t2 = ln2p.tile([128, D], F32, name="t2b", tag="t2b")
                            nc.vector.tensor_add(t2[:], x1[i][:], bb2_sb[:])
                            xo = ln2p.tile([128, D], F32, name="xo", tag="xo")
                            nc.vector.tensor_add(xo[:], t1[:], t2[:])
                            nc.sync.dma_start(out=out[i * 128:(i + 1) * 128, :], in_=xo[:])
                ln2c.release()
                bp.release()
                wstr5.release()
                wstr4.release()

    nc.finalize()
    return nc


_NC = None


def _get_nc():
    global _NC
    if _NC is None:
        _NC = _build()
    return _NC


def _host_prep(batch_X, padding_mask, W_Q, W_K, W_V, W_O, W1, b1, W2, b2,
               gamma1, beta1, gamma2, beta2):
    """Build the 8 per-core input maps."""
    f = np.float32
    X = np.asarray(batch_X, f)
    pm = np.asarray(padding_mask)

    def colblocks(w, nt):
        # [D, nt*128] -> [nt, 128, D]: block t, partition p holds the
        # contraction row for W[:, t*128+m] laid out [c, m] contiguously,
        # so each DMA partition line is one contiguous 4KB read
        nd = w.shape[0] // 128
        return np.ascontiguousarray(
            np.asarray(w, f).reshape(nd, 128, nt, 128).transpose(2, 1, 0, 3)
        ).reshape(nt, 128, w.shape[0])

    shared = {
        "wqb": colblocks(W_Q, NDC),
        "wkb": colblocks(W_K, NDC),
        "wv": np.ascontiguousarray(np.asarray(W_V, f)),
        "wo": np.ascontiguousarray(np.asarray(W_O, f)),
        "w1b": colblocks(W1, NFT),
        "w2": np.ascontiguousarray(np.asarray(W2, f)),
        "b1m": np.ascontiguousarray(np.asarray(b1, f).reshape(NFT, 128).T),
        "gb1": np.ascontiguousarray(
            np.broadcast_to(np.asarray(gamma1, f), (128, D))),
        "bb1": np.ascontiguousarray(
            np.broadcast_to(np.asarray(beta1, f), (128, D))),
        "gb2": np.ascontiguousarray(
            np.broadcast_to(np.asarray(gamma2, f), (128, D))),
        "bb2": np.ascontiguousarray(
            np.broadcast_to(np.asarray(beta2, f), (128, D))),
        "b2b": np.ascontiguousarray(
            np.broadcast_to(np.asarray(b2, f), (128, D))),
    }
    in_maps = []
    for core in range(NCORES):
        b = core // (NCORES // B)
        q0 = (core % (NCORES // B)) * Q
        xb = X[b]
        keep_f = (pm[b, q0:q0 + Q] != 0).astype(f)
        m = dict(shared)
        m["xt"] = np.ascontiguousarray(xb.T)
        m["xtq"] = np.ascontiguousarray(xb[q0:q0 + Q].T)
        m["xq"] = np.ascontiguousarray(xb[q0:q0 + Q])
        m["keep"] = np.ascontiguousarray(keep_f.reshape(NQT, 128).T)
        in_maps.append(m)
    return in_maps


def kernel(**inputs):
    nc = _get_nc()
    in_maps = _host_prep(**inputs)
    res = bass_utils.run_bass_kernel_spmd(nc, in_maps, list(range(NCORES)))
    out = np.empty((B, S, D), np.float32)
    for core in range(NCORES):
        b = core // (NCORES // B)
        q0 = (core % (NCORES // B)) * Q
        out[b, q0:q0 + Q] = res.results[core]["out"]
    return out
